# revision 1
# baseline (speedup 1.0000x reference)
"""AttnBlock (GroupNorm + single-head spatial attention + proj + residual)
for Trainium2, SPMD across 8 NeuronCores.

Sharding: data-parallel over batch (4 images) x 2-way split of query
positions per image => 8 cores.  Attention is computed per-image with the
full key/value set on every core, so there are no collectives.

Per-core algorithm (image b, query half h):
  - Spatial positions of the local image copy are rolled so the core's
    2048 query positions are always local positions [0, 2048).  Attention
    and GroupNorm are permutation-invariant over spatial positions, so the
    roll is transparent; the host un-rolls when assembling the output.
  - GroupNorm is folded into the projections: h = a*x + b (per channel,
    a/b derived on device from bn_stats), so q/k/v matmuls consume raw x
    with per-channel-scaled weights.
  - wproj is folded into the v projection on the host (softmax rows sum
    to one, so  Wp(V P) + bp == (Wp V) P + bp), removing the output
    projection and its transposes entirely.
  - Scores are computed transposed (sT[j, i] = k.q), softmax over the
    partition-tiled j axis with no max subtraction (scores are O(5) so
    exp cannot overflow), and the sum-of-exp denominator comes for free
    as a ones-column of v'T in the PV matmul (padded to 258 cols: f32r
    matmuls need an even moving size).
  - k's projection bias is dropped: q_i . bke is constant over the
    softmax axis, so it cancels; q keeps its (GroupNorm-folded) bias.
  - Projections run as float32r (TF32-like mantissa, full PE rate at
    N>=256); the attention q/k/e/v operands are bf16 so LDWEIGHTS uses
    the fast-weight-load path (the PV matmuls are weight-load-bound).
  - PE warm-up matmuls fill the initial DMA/stats wait so the HAM clock
    gate is released before the real matmul stream starts.
"""

import numpy as np

import concourse.bacc as bacc
import concourse.bass as bass
import concourse.mybir as mybir
import concourse.tile as tile
from concourse.tile import add_dep_helper
from concourse.bass_utils import run_bass_kernel_spmd

F32 = mybir.dt.float32
F32R = mybir.dt.float32r
BF16 = mybir.dt.bfloat16

C = 256          # channels
HW = 4096        # spatial positions (64*64)
B = 4            # batch
NCORES = 8
IH = HW // 2     # query positions per core
P = 128          # partitions
NCC = C // P     # channel chunks (2)
IBLK = 512       # query i-block (scores moving free dim)
NIB = IH // IBLK # 4 i-blocks per core
NJT = HW // P    # 32 key tiles
NSUB = IBLK // P # 4 i-subtiles per block
EPS = 1e-6
SCALE = 1.0 / 16.0  # 1/sqrt(C)

_PROGRAM = None  # cached (nc)
LAST_RESULTS = None  # BassKernelResults of the most recent run (for test harness)
TRACE = False


def _round_f32r(x):
    u = np.ascontiguousarray(x, dtype=np.float32).view(np.uint32)
    r = ((u.astype(np.uint64) + 0x800) & 0xFFFFF000).astype(np.uint32)
    return r.view(np.float32)


def _build_program(reps=1):
    nc = bacc.Bacc()

    xr_d = nc.declare_dram_parameter("xr", [C, HW], F32R, isOutput=False)
    xth_d = nc.declare_dram_parameter("xth", [IH, C], F32, isOutput=False)
    wq_d = nc.declare_dram_parameter("wqt", [C, C], F32R, isOutput=False)
    wk_d = nc.declare_dram_parameter("wkt", [C, C], F32R, isOutput=False)
    w2_d = nc.declare_dram_parameter("w2t", [C, C], F32R, isOutput=False)
    bq_d = nc.declare_dram_parameter("bq", [C], F32, isOutput=False)
    bk_d = nc.declare_dram_parameter("bk", [C], F32, isOutput=False)
    b2h_d = nc.declare_dram_parameter("b2h", [C], F32, isOutput=False)  # wproj@bv+bproj
    gns_d = nc.declare_dram_parameter("gns", [C], F32, isOutput=False)
    gnb_d = nc.declare_dram_parameter("gnb", [C], F32, isOutput=False)
    out_d = nc.declare_dram_parameter("out", [IH, C], F32, isOutput=True)

    b2_dram = nc.dram_tensor("b2_bounce", [C], F32)

    with tile.TileContext(nc) as tc:
      for _rep in range(reps):
        with (
            tc.tile_pool(name="wt", bufs=1) as wt,
            tc.tile_pool(name="xp", bufs=1) as xp,
            tc.tile_pool(name="qkv", bufs=1) as qkv,
            tc.tile_pool(name="scr", bufs=2) as scr,
        ):
            # ---------- constants ----------
            G = wt.tile([P, P], F32, tag="G", name="G")
            nc.gpsimd.memset(G, 0.0)
            nc.gpsimd.memset(G[0:64, 0:64], 1.0 / 64.0)
            nc.gpsimd.memset(G[64:128, 64:128], 1.0 / 64.0)
            eps_t = wt.tile([P, 1], F32, tag="eps", name="eps")
            nc.vector.memset(eps_t, EPS)

            # ---------- x loads first (startup critical path) ----------
            # issue from four sequencers in parallel: one dma_start costs
            # ~0.6us of sequencer issue time, and x is the critical path
            xr_sb = [xp.tile([P, HW], F32R, tag=f"xr{cc}", name=f"xr{cc}")
                     for cc in range(NCC)]
            _eng = [nc.sync, nc.scalar, nc.gpsimd]
            for w in range(8):
                for cc in range(NCC):
                    _eng[(w * NCC + cc) % 3].dma_start(
                        out=xr_sb[cc][:, w * 512:(w + 1) * 512],
                        in_=xr_d[cc * P:(cc + 1) * P, w * 512:(w + 1) * 512],
                    )

            # ---------- load weights / params ----------
            w_sb = {}
            for name, d in (("q", wq_d), ("k", wk_d), ("v", w2_d)):
                for cc in range(NCC):
                    t = wt.tile([P, C], F32R, tag=f"w{name}{cc}", name=f"w{name}{cc}")
                    nc.scalar.dma_start(out=t, in_=d[cc * P:(cc + 1) * P, :])
                    w_sb[name, cc] = t
            par_sb = {}
            for name, d in (("bq", bq_d), ("bk", bk_d), ("gns", gns_d), ("gnb", gnb_d)):
                for cc in range(NCC):
                    t = wt.tile([P, 1], F32, tag=f"{name}{cc}", name=f"{name}{cc}")
                    nc.scalar.dma_start(out=t, in_=d[cc * P:(cc + 1) * P].unsqueeze(1))
                    par_sb[name, cc] = t
            b2h_sb = wt.tile([1, C], F32, tag="b2h", name="b2h")
            nc.sync.dma_start(out=b2h_sb, in_=b2h_d[:].unsqueeze(0))

            # ---------- residual (needed only at epilogue; last in DMA order) ----------
            xth_sb = xp.tile([P, IH // P, C], F32, tag="xth", name="xth")
            xth_dmas = []
            for s in range(IH // P):
                xth_dmas.append(nc.sync.dma_start(out=xth_sb[:, s, :], in_=xth_d[s * P:(s + 1) * P, :]))

            # ---------- GroupNorm stats (on rounded x; error ~1e-7) ----------
            with tc.tile_pool(name="psA", bufs=2, space="PSUM") as psA:
                # PE warm-up while x DMA + stats run: fills idle time and
                # brings HAM out of the cold 1.2 GHz state before real work
                warm_ps = psA.tile([P, 128], F32, tag="warm", name="warm")
                warm_rhs = wt.tile([P, 128], F32, tag="warm_rhs", name="warm_rhs")
                nc.gpsimd.memset(warm_rhs, 0.0)
                for _ in range(36):
                    nc.tensor.matmul(warm_ps, G, warm_rhs, start=True, stop=True)
                a_sb, b_sb = [], []
                st6s = [scr.tile([P, 8, 6], F32, tag=f"st6{cc}", name=f"st6{cc}")
                        for cc in range(NCC)]
                last_bn = None
                for w in range(8):
                    for cc in range(NCC):
                        last_bn = nc.vector.bn_stats(out=st6s[cc][:, w, :], in_=xr_sb[cc][:, w * 512:(w + 1) * 512])
                for _d in xth_dmas:
                    add_dep_helper(_d.ins, last_bn.ins, sync=True,
                                   reason="defer residual load until stats read x")
                for cc in range(NCC):
                    st6 = st6s[cc]
                    mv = scr.tile([P, 2], F32, tag="mv", name="mv")
                    nc.vector.bn_aggr(out=mv, in_=st6)
                    st3 = scr.tile([P, 3], F32, tag="st3", name="st3")
                    nc.vector.tensor_copy(st3[:, 0:2], mv)
                    nc.vector.tensor_mul(st3[:, 2:3], mv[:, 0:1], mv[:, 0:1])
                    gp = psA.tile([P, 3], F32, tag="gp", name="gp")
                    nc.tensor.matmul(gp, G, st3, start=True, stop=True)
                    # group stats, broadcast per channel: mean, E[var], E[mean^2]
                    gs = scr.tile([P, 3], F32, tag="gs", name="gs")
                    nc.vector.tensor_copy(gs, gp)
                    t1 = scr.tile([P, 1], F32, tag="t1", name="t1")
                    nc.vector.tensor_mul(t1, gs[:, 0:1], gs[:, 0:1])
                    vg = scr.tile([P, 1], F32, tag="vg", name="vg")
                    nc.vector.tensor_add(vg, gs[:, 1:2], gs[:, 2:3])
                    nc.vector.tensor_sub(vg, vg, t1)
                    sd = scr.tile([P, 1], F32, tag="sd", name="sd")
                    nc.scalar.activation(out=sd, in_=vg, func=mybir.ActivationFunctionType.Sqrt, bias=eps_t)
                    rstd = scr.tile([P, 1], F32, tag="rstd", name="rstd")
                    nc.vector.reciprocal(rstd, sd)
                    a_t = wt.tile([P, 1], F32, tag=f"a{cc}", name=f"a{cc}")
                    nc.vector.tensor_mul(a_t, rstd, par_sb["gns", cc])
                    t2 = scr.tile([P, 1], F32, tag="t2", name="t2")
                    nc.vector.tensor_mul(t2, gs[:, 0:1], a_t)
                    b_t = wt.tile([P, 1], F32R, tag=f"b{cc}", name=f"b{cc}")
                    nc.vector.tensor_sub(b_t, par_sb["gnb", cc], t2)
                    a_sb.append(a_t)
                    b_sb.append(b_t)

                for _ in range(20):
                    nc.tensor.matmul(warm_ps, G, warm_rhs, start=True, stop=True)

                # ---------- fold GroupNorm scale into weights ----------
                wf = {}
                for name in ("q", "k", "v"):
                    for cc in range(NCC):
                        t = wt.tile([P, C], F32R, tag=f"wf{name}{cc}", name=f"wf{name}{cc}")
                        nc.vector.tensor_scalar_mul(t, w_sb[name, cc], a_sb[cc])
                        wf[name, cc] = t

                # ---------- effective biases ----------
                be = {}
                for name in ("q",):
                    for cc in range(NCC):
                        bp = psA.tile([P, 1], F32, tag="bp", name="bp")
                        nc.tensor.matmul(bp, w_sb[name, 0][:, cc * P:(cc + 1) * P].bitcast(F32), b_sb[0].bitcast(F32), start=True, stop=False)
                        nc.tensor.matmul(bp, w_sb[name, 1][:, cc * P:(cc + 1) * P].bitcast(F32), b_sb[1].bitcast(F32), start=False, stop=True)
                        t = wt.tile([P, 1], F32, tag=f"be{name}{cc}", name=f"be{name}{cc}")
                        nc.vector.tensor_add(t, bp, par_sb["b" + name, cc])
                        be[name, cc] = t
                b2p = psA.tile([1, C], F32, tag="b2p", name="b2p")
                nc.tensor.matmul(b2p, b_sb[0].bitcast(F32), w_sb["v", 0].bitcast(F32), start=True, stop=False)
                nc.tensor.matmul(b2p, b_sb[1].bitcast(F32), w_sb["v", 1].bitcast(F32), start=False, stop=True)
                b2row = wt.tile([1, C], F32, tag="b2row", name="b2row")
                nc.vector.tensor_add(b2row, b2p, b2h_sb)
                nc.sync.dma_start(out=b2_dram[:].unsqueeze(0), in_=b2row)
                b2bc = wt.tile([P, C], F32, tag="b2bc", name="b2bc")
                nc.sync.dma_start(
                    out=b2bc,
                    in_=bass.AP(tensor=b2_dram, offset=0, ap=[[0, P], [1, C]]),
                )

            # ---------- projections ----------
            q_sb = [qkv.tile([P, IH], BF16, tag=f"q{cc}", name=f"q{cc}") for cc in range(NCC)]
            k_sb = [qkv.tile([P, HW], BF16, tag=f"k{cc}", name=f"k{cc}") for cc in range(NCC)]
            vT_sb = qkv.tile([P, NJT, C + 2], BF16, tag="vT", name="vT")
            ones_t = wt.tile([P, 2], F32, tag="ones", name="ones")
            nc.vector.memset(ones_t, 1.0)
            for jt in range(NJT):
                nc.vector.tensor_copy(vT_sb[:, jt, C:C + 2], ones_t)

            with tc.tile_pool(name="psB", bufs=3, space="PSUM") as psB:
                for cc in range(NCC):
                    for ib in range(NIB):
                        pq = psB.tile([P, IBLK], F32, tag="pq", name="pq")
                        sl = slice(ib * IBLK, (ib + 1) * IBLK)
                        nc.tensor.matmul(pq, wf["q", 0][:, cc * P:(cc + 1) * P], xr_sb[0][:, sl], start=True, stop=False)
                        nc.tensor.matmul(pq, wf["q", 1][:, cc * P:(cc + 1) * P], xr_sb[1][:, sl], start=False, stop=True)
                        nc.vector.tensor_scalar_add(q_sb[cc][:, sl], pq, be["q", cc])
                for cc in range(NCC):
                    for ib in range(HW // IBLK):
                        pk = psB.tile([P, IBLK], F32, tag="pq", name="pq")
                        sl = slice(ib * IBLK, (ib + 1) * IBLK)
                        nc.tensor.matmul(pk, wf["k", 0][:, cc * P:(cc + 1) * P], xr_sb[0][:, sl], start=True, stop=False)
                        nc.tensor.matmul(pk, wf["k", 1][:, cc * P:(cc + 1) * P], xr_sb[1][:, sl], start=False, stop=True)
                        # k's bias only adds a j-constant to each softmax row
                        # (q_i . bke), so it is dropped; plain copy on ACT
                        nc.scalar.copy(k_sb[cc][:, sl], pk)
                for jt in range(NJT):
                    pv = psB.tile([P, C], F32, tag="pv", name="pv")
                    sl = slice(jt * P, (jt + 1) * P)
                    nc.tensor.matmul(pv, xr_sb[0][:, sl], wf["v", 0], start=True, stop=False)
                    nc.tensor.matmul(pv, xr_sb[1][:, sl], wf["v", 1], start=False, stop=True)
                    # add (bias-folded) b2 into v'; softmax weights sum to 1 so
                    # this equals adding it after normalization
                    nc.vector.tensor_add(vT_sb[:, jt, 0:C], pv, b2bc)

            # ---------- attention ----------
            with (
                tc.tile_pool(name="psS", bufs=3, space="PSUM") as psS,
                tc.tile_pool(name="psAT", bufs=5, space="PSUM") as psAT,
                tc.tile_pool(name="eP", bufs=3) as eP,
                tc.tile_pool(name="oP", bufs=3) as oP,
                tc.tile_pool(name="rP", bufs=4) as rP,
            ):
                blocks = [(0, IBLK), (IBLK, IBLK), (2 * IBLK, IBLK),
                          (3 * IBLK, IBLK // 2), (3 * IBLK + IBLK // 2, IBLK // 2)]
                for i0, ilen in blocks:
                    isl = slice(i0, i0 + ilen)
                    nsub = ilen // P
                    at = [psAT.tile([P, C + 2], F32, tag="at", name="at") for _ in range(nsub)]
                    sps = {}

                    def scores(jt):
                        jsl = slice(jt * P, (jt + 1) * P)
                        sp = psS.tile([P, ilen], F32, tag="sp", name="sp")
                        nc.tensor.matmul(sp, k_sb[0][:, jsl], q_sb[0][:, isl], start=True, stop=False)
                        nc.tensor.matmul(sp, k_sb[1][:, jsl], q_sb[1][:, isl], start=False, stop=True)
                        sps[jt] = sp

                    scores(0)
                    scores(1)
                    for jt in range(NJT):
                        eT = eP.tile([P, ilen], BF16, tag="eT", name="eT")
                        nc.scalar.activation(out=eT, in_=sps.pop(jt), func=mybir.ActivationFunctionType.Exp, scale=SCALE)
                        if jt + 2 < NJT:
                            scores(jt + 2)
                        for s in range(nsub):
                            nc.tensor.matmul(
                                at[s], eT[:, s * P:(s + 1) * P], vT_sb[:, jt, :],
                                start=(jt == 0), stop=(jt == NJT - 1),
                            )
                    for s in range(nsub):
                        g = i0 // P + s
                        rec = rP.tile([P, 1], F32, tag="rec", name="rec")
                        nc.vector.reciprocal(rec, at[s][:, C:C + 1])
                        ot = oP.tile([P, C], F32, tag="ot", name="ot")
                        nc.vector.tensor_scalar_mul(ot, at[s][:, 0:C], rec)
                        nc.vector.tensor_add(ot, ot, xth_sb[:, g, :])
                        nc.sync.dma_start(out=out_d[g * P:(g + 1) * P, :], in_=ot)

    nc.finalize()
    return nc


def _get_program():
    global _PROGRAM
    if _PROGRAM is None:
        _PROGRAM = _build_program()
    return _PROGRAM


def kernel(x, gn_scale, gn_bias, wq, bq, wk, bk, wv, bv, wproj, bproj):
    global LAST_RESULTS
    x = np.asarray(x, dtype=np.float32)
    gn_scale = np.asarray(gn_scale, dtype=np.float32)
    gn_bias = np.asarray(gn_bias, dtype=np.float32)
    wq_ = np.asarray(wq, dtype=np.float32)
    wk_ = np.asarray(wk, dtype=np.float32)
    wv_ = np.asarray(wv, dtype=np.float32)
    wp_ = np.asarray(wproj, dtype=np.float32)
    bq_ = np.asarray(bq, dtype=np.float32)
    bk_ = np.asarray(bk, dtype=np.float32)
    bv_ = np.asarray(bv, dtype=np.float32)
    bp_ = np.asarray(bproj, dtype=np.float32)

    b, c, h, w = x.shape
    assert (b, c, h * w) == (B, C, HW), x.shape

    w2 = (wp_.astype(np.float64) @ wv_.astype(np.float64)).astype(np.float32)
    b2h = (wp_.astype(np.float64) @ bv_.astype(np.float64)).astype(np.float32) + bp_

    wqt = _round_f32r(np.ascontiguousarray(wq_.T))
    wkt = _round_f32r(np.ascontiguousarray(wk_.T))
    w2t = _round_f32r(np.ascontiguousarray(w2.T))

    xf = x.reshape(B, C, HW)
    in_maps = []
    for core in range(NCORES):
        bi, hi = core // 2, core % 2
        xi = np.roll(xf[bi], -IH * hi, axis=1)
        in_maps.append({
            "xr": _round_f32r(xi),
            "xth": np.ascontiguousarray(xi[:, :IH].T),
            "wqt": wqt, "wkt": wkt, "w2t": w2t,
            "bq": bq_, "bk": bk_, "b2h": b2h,
            "gns": gn_scale, "gnb": gn_bias,
        })

    nc = _get_program()
    res = run_bass_kernel_spmd(nc, in_maps, list(range(NCORES)), trace=TRACE)
    LAST_RESULTS = res

    out = np.empty((B, C, HW), dtype=np.float32)
    for core in range(NCORES):
        bi, hi = core // 2, core % 2
        out[bi][:, hi * IH:(hi + 1) * IH] = res.results[core]["out"].T
    return out.reshape(B, C, h, w)

